# revision 8
# baseline (speedup 1.0000x reference)
"""Trainium2 Bass kernel for nn_JointTag_withfeature (ragged_sequence).

Reference computation (per batch row):
  cls_rep = seq[:, 0, :]                      # [B, D]
  valid_out = compact_valid(seq, valid_ids)   # valid tokens packed front, rest 0
  enriched = relu(feat @ enr_w + enr_b)       # original token order
  h = relu(concat([valid_out, enriched], -1) @ w1 + b1)
  tag_logits = h @ w2 + b2
  cls_logits = cls_rep @ cls_w + cls_b

Strategy: pure data-parallel over 8 NeuronCores (8 batch rows per core).
On-device, the ragged compaction is expressed as a matmul with a 0/1
selection matrix P^T[l, j] = (valid[l] and rank[l] == j), built from a
triangular-ones cumsum matmul plus an is_equal against an iota row.
seq^T-compacted = seq.T-contract-over-l with P^T, which simultaneously
performs the compaction AND yields channel-major activations for the
main GEMM chain (all later matmuls contract over the channel dim).
"""

import numpy as np

import concourse.bass as bass
import concourse.bacc as bacc
import concourse.mybir as mybir
import concourse.tile as tile
from concourse.bass_utils import run_bass_kernel_spmd

F32 = mybir.dt.float32
BF16 = mybir.dt.bfloat16
I32 = mybir.dt.int32
NP_BF16 = mybir.dt.np(mybir.dt.bfloat16)
AF = mybir.ActivationFunctionType
OP = mybir.AluOpType

N_CORES = 8
B, L, D = 64, 512, 768
NF = 100
WAIST = 768
TOK = 3
CLS = 2
BL = B // N_CORES          # batch rows per core = 8
P = 128
LC = L // P                # 4 token chunks per row
DC = D // P                # 6 channel chunks
WC = WAIST // P            # 6 waist chunks
KC = DC + 1                # contraction chunks of w1 (6x128 seq + 1x100 feat)


def build_program():
    nc = bacc.Bacc("TRN2", target_bir_lowering=False, debug=False)

    seq = nc.dram_tensor("seq", [BL, L, D], F32, kind="ExternalInput").ap()
    feat = nc.dram_tensor("feat", [BL, L, NF], F32, kind="ExternalInput").ap()
    valid = nc.dram_tensor("valid", [BL, L], I32, kind="ExternalInput").ap()
    enr_w = nc.dram_tensor("enr_w", [NF, NF], BF16, kind="ExternalInput").ap()
    enr_b = nc.dram_tensor("enr_b", [NF], F32, kind="ExternalInput").ap()
    w1 = nc.dram_tensor("w1", [D + NF, WAIST], BF16, kind="ExternalInput").ap()
    b1 = nc.dram_tensor("b1", [WAIST], F32, kind="ExternalInput").ap()
    w2 = nc.dram_tensor("w2", [WAIST, TOK], BF16, kind="ExternalInput").ap()
    b2 = nc.dram_tensor("b2", [TOK], F32, kind="ExternalInput").ap()
    cls_w = nc.dram_tensor("cls_w", [D, CLS], F32, kind="ExternalInput").ap()
    cls_b = nc.dram_tensor("cls_b", [CLS], F32, kind="ExternalInput").ap()

    tag = nc.dram_tensor("tag", [BL, L, TOK], F32, kind="ExternalOutput").ap()
    cls = nc.dram_tensor("cls", [BL, CLS], F32, kind="ExternalOutput").ap()

    # NEFF-embedded constants
    ident_np = np.eye(P, dtype=np.float32)
    iota_np = np.broadcast_to(
        np.arange(L, dtype=np.float32), (P, L)
    ).copy()
    tri_np = np.triu(np.ones((L, L), dtype=np.float32))  # U[k, l] = 1 iff k <= l
    ident_d = nc.inline_tensor(ident_np, name="identc").ap()
    identb_d = nc.inline_tensor(ident_np.astype(NP_BF16), name="identbc").ap()
    iota_d = nc.inline_tensor(iota_np, name="iotac").ap()
    tri_d = nc.inline_tensor(tri_np.astype(NP_BF16), name="tric").ap()

    with tile.TileContext(nc) as tc:
        with (
            tc.tile_pool(name="const", bufs=1) as cp,
            tc.tile_pool(name="work", bufs=2) as wp,
            tc.tile_pool(name="ps_big", bufs=2, space="PSUM") as pb,
            tc.tile_pool(name="ps_small", bufs=1, space="PSUM") as ps,
        ):
            # ---- persistent constants / weights in SBUF ----
            ident = cp.tile([P, P], F32)
            nc.sync.dma_start(ident[:], ident_d[:])
            identb = cp.tile([P, P], BF16)
            nc.sync.dma_start(identb[:], identb_d[:])
            iota = cp.tile([P, L], F32)
            nc.sync.dma_start(iota[:], iota_d[:])
            tri = cp.tile([P, LC, L], BF16)
            nc.sync.dma_start(tri[:], tri_d.rearrange("(c p) l -> p c l", p=P))

            w1_sb = cp.tile([P, KC, WAIST], BF16)
            nc.sync.dma_start(
                w1_sb[:, :DC, :], w1[:D].rearrange("(c p) w -> p c w", p=P)
            )
            nc.sync.dma_start(w1_sb[:NF, DC, :], w1[D:])
            b1T = cp.tile([P, WC], F32)
            nc.sync.dma_start(b1T[:], b1.rearrange("(c p) -> p c", p=P))

            w2_sb = cp.tile([P, WC, TOK], BF16)
            nc.sync.dma_start(w2_sb[:], w2.rearrange("(c p) t -> p c t", p=P))
            b2T = cp.tile([TOK, 1], F32)
            nc.sync.dma_start(b2T[:], b2.rearrange("(t o) -> t o", o=1))

            clsw_sb = cp.tile([P, DC, CLS], F32)
            nc.sync.dma_start(clsw_sb[:], cls_w.rearrange("(c p) t -> p c t", p=P))
            clsbT = cp.tile([CLS, 1], F32)
            nc.sync.dma_start(clsbT[:], cls_b.rearrange("(t o) -> t o", o=1))

            enrw_sb = cp.tile([NF, NF], BF16)
            nc.sync.dma_start(enrw_sb[:], enr_w[:])
            enrbT = cp.tile([NF, 1], F32)
            nc.sync.dma_start(enrbT[:], enr_b.rearrange("(f o) -> f o", o=1))

            # ---- per-core prologue: destination slots tgtT[l, b] ----
            # cumsumT[l, b] = sum_k tri[k, l] * valid[b, k]  (inclusive cumsum)
            # tgtT = (cumsumT - 1) if valid else -1000
            vT_i = cp.tile([P, LC, BL], I32)
            valid_t = valid.rearrange("b (c p) -> p c b", p=P)
            for lc in range(LC):
                nc.sync.dma_start(vT_i[:, lc, :], valid_t[:, lc, :])
            vT_f = cp.tile([P, LC, BL], BF16)
            nc.vector.tensor_copy(vT_f[:], vT_i[:])

            tgtT = cp.tile([P, LC, BL], F32)
            for lc in range(LC):
                pcs = ps.tile([P, BL], F32, tag="pft")
                for kc in range(LC):
                    nc.tensor.matmul(
                        pcs[:],
                        tri[:, kc, lc * P:(lc + 1) * P],
                        vT_f[:, kc, :],
                        start=(kc == 0),
                        stop=(kc == LC - 1),
                    )
                nc.vector.tensor_scalar_add(tgtT[:, lc, :], pcs[:], 999.0)
                nc.vector.tensor_tensor(
                    out=tgtT[:, lc, :], in0=tgtT[:, lc, :], in1=vT_f[:, lc, :],
                    op=OP.mult,
                )
                nc.vector.tensor_scalar_add(tgtT[:, lc, :], tgtT[:, lc, :], -1000.0)

            # ---- cls head (once per core) ----
            clsT = cp.tile([P, DC, BL], F32)
            cls_rep_t = seq[:, 0, :].rearrange("b (c p) -> p c b", p=P)
            for dc in range(DC):
                nc.sync.dma_start(clsT[:, dc, :], cls_rep_t[:, dc, :])
            pcls = ps.tile([CLS, BL], F32, tag="penr")
            for dc in range(DC):
                nc.tensor.matmul(
                    pcls[:], clsw_sb[:, dc, :], clsT[:, dc, :],
                    start=(dc == 0), stop=(dc == DC - 1),
                )
            clsv = cp.tile([CLS, BL], F32)
            nc.scalar.activation(clsv[:], pcls[:], AF.Identity, bias=clsbT[:, 0:1])
            pct = ps.tile([BL, CLS], F32, tag="ptt")
            nc.tensor.transpose(pct[:], clsv[:], ident[:CLS, :CLS])
            clso = cp.tile([BL, CLS], F32)
            nc.vector.tensor_copy(clso[:], pct[:])
            nc.sync.dma_start(cls[:], clso[:])

            # ---- main per-row pipeline ----
            for b in range(BL):
                seq_row = seq[b].rearrange("(c p) d -> p c d", p=P)
                seq_bf = []
                for lc in range(LC):
                    sf = wp.tile([P, D], F32, tag=f"seqf{lc}")
                    nc.sync.dma_start(sf[:], seq_row[:, lc, :])
                    sb = wp.tile([P, D], BF16, tag=f"seq{lc}")
                    nc.gpsimd.tensor_copy(sb[:], sf[:])
                    seq_bf.append(sb)
                feat_f = wp.tile([P, LC, NF], F32, tag="featf")
                nc.sync.dma_start(feat_f[:], feat[b].rearrange("(c p) f -> p c f", p=P))
                feat_sb = wp.tile([P, LC, NF], BF16, tag="feat")
                nc.scalar.copy(feat_sb[:], feat_f[:])

                # selection matrix P^T[l, j] = (tgtT[l] == j)
                pt_sb = wp.tile([P, LC, L], BF16, tag="pt")
                for lc in range(LC):
                    nc.vector.tensor_scalar(
                        pt_sb[:, lc, :], iota[:], tgtT[:, lc, b:b + 1], None,
                        op0=OP.is_equal,
                    )

                # featT via PE transpose, then enriched^T = relu(enr_w^T @ featT)
                pft = ps.tile([NF, L], BF16, tag="pft")
                for lc in range(LC):
                    nc.tensor.transpose(
                        pft[:, lc * P:(lc + 1) * P], feat_sb[:, lc, :], identb[:]
                    )
                featT = wp.tile([NF, L], BF16, tag="featT")
                nc.vector.tensor_copy(featT[:], pft[:])

                penr = ps.tile([NF, L], F32, tag="penr")
                nc.tensor.matmul(penr[:], enrw_sb[:], featT[:], start=True, stop=True)
                enrT = wp.tile([NF, L], BF16, tag="enrT")
                nc.scalar.activation(enrT[:], penr[:], AF.Relu, bias=enrbT[:, 0:1])

                # compacted seq^T: xT[d, j] = sum_l seq[l, d] * P^T[l, j]
                xT = wp.tile([P, DC, L], BF16, tag="xT")
                for dc in range(DC):
                    px = pb.tile([P, L], F32, tag="px")
                    for lc in range(LC):
                        nc.tensor.matmul(
                            px[:],
                            seq_bf[lc][:, dc * P:(dc + 1) * P],
                            pt_sb[:, lc, :],
                            start=(lc == 0),
                            stop=(lc == LC - 1),
                        )
                    if dc % 2 == 0:
                        nc.vector.tensor_copy(xT[:, dc, :], px[:])
                    else:
                        nc.scalar.copy(xT[:, dc, :], px[:])

                # hT = relu(w1^T @ concat(xT, enrT) + b1)
                hT = wp.tile([P, WC, L], BF16, tag="hT")
                for wc in range(WC):
                    ph = pb.tile([P, L], F32, tag="ph")
                    for dc in range(DC):
                        nc.tensor.matmul(
                            ph[:],
                            w1_sb[:, dc, wc * P:(wc + 1) * P],
                            xT[:, dc, :],
                            start=(dc == 0),
                            stop=False,
                        )
                    nc.tensor.matmul(
                        ph[:],
                        w1_sb[:NF, DC, wc * P:(wc + 1) * P],
                        enrT[:],
                        start=False,
                        stop=True,
                    )
                    nc.scalar.activation(
                        hT[:, wc, :], ph[:], AF.Relu, bias=b1T[:, wc:wc + 1]
                    )

                # tagT = w2^T @ hT + b2, then transpose back to token-major
                pt_ps = ps.tile([TOK, L], F32, tag="pt_ps")
                for wc in range(WC):
                    nc.tensor.matmul(
                        pt_ps[:], w2_sb[:, wc, :], hT[:, wc, :],
                        start=(wc == 0), stop=(wc == WC - 1),
                    )
                tagT_sb = wp.tile([TOK, L], F32, tag="tagT")
                nc.scalar.activation(tagT_sb[:], pt_ps[:], AF.Identity, bias=b2T[:, 0:1])

                tag_sb = wp.tile([P, LC, TOK], F32, tag="tago")
                for lc in range(LC):
                    ptt = ps.tile([P, TOK], F32, tag="ptt")
                    nc.tensor.transpose(
                        ptt[:], tagT_sb[:, lc * P:(lc + 1) * P], ident[:TOK, :TOK]
                    )
                    nc.vector.tensor_copy(tag_sb[:, lc, :], ptt[:])
                nc.sync.dma_start(tag[b].rearrange("(c p) t -> p c t", p=P), tag_sb[:])

    nc.compile()
    return nc


_NC = None


def kernel(sequence_output, features, valid_ids, enr_w, enr_b, w1, b1, w2, b2,
           cls_w, cls_b):
    global _NC
    if _NC is None:
        _NC = build_program()

    shared = {
        "enr_w": np.ascontiguousarray(enr_w).astype(NP_BF16),
        "enr_b": np.ascontiguousarray(enr_b, np.float32),
        "w1": np.ascontiguousarray(w1).astype(NP_BF16),
        "b1": np.ascontiguousarray(b1, np.float32),
        "w2": np.ascontiguousarray(w2).astype(NP_BF16),
        "b2": np.ascontiguousarray(b2, np.float32),
        "cls_w": np.ascontiguousarray(cls_w, np.float32),
        "cls_b": np.ascontiguousarray(cls_b, np.float32),
    }
    in_maps = []
    for c in range(N_CORES):
        sl = slice(c * BL, (c + 1) * BL)
        in_maps.append({
            "seq": np.ascontiguousarray(sequence_output[sl], np.float32),
            "feat": np.ascontiguousarray(features[sl], np.float32),
            "valid": np.ascontiguousarray(valid_ids[sl], np.int32),
            **shared,
        })

    res = run_bass_kernel_spmd(_NC, in_maps, list(range(N_CORES)))
    tag = np.concatenate([res.results[c]["tag"] for c in range(N_CORES)], axis=0)
    cls = np.concatenate([res.results[c]["cls"] for c in range(N_CORES)], axis=0)
    return (cls.astype(np.float32), tag.astype(np.float32))


# revision 9
# speedup vs baseline: 1.0330x; 1.0330x over previous
"""Trainium2 Bass kernel for nn_JointTag_withfeature (ragged_sequence).

Reference computation (per batch row):
  cls_rep = seq[:, 0, :]                      # [B, D]
  valid_out = compact_valid(seq, valid_ids)   # valid tokens packed front, rest 0
  enriched = relu(feat @ enr_w + enr_b)       # original token order
  h = relu(concat([valid_out, enriched], -1) @ w1 + b1)
  tag_logits = h @ w2 + b2
  cls_logits = cls_rep @ cls_w + cls_b

Strategy: pure data-parallel over 8 NeuronCores (8 batch rows per core).
On-device, the ragged compaction is expressed as a matmul with a 0/1
selection matrix P^T[l, j] = (valid[l] and rank[l] == j), built from a
triangular-ones cumsum matmul plus an is_equal against an iota row.
seq^T-compacted = seq.T-contract-over-l with P^T, which simultaneously
performs the compaction AND yields channel-major activations for the
main GEMM chain (all later matmuls contract over the channel dim).
"""

import numpy as np

import concourse.bass as bass
import concourse.bacc as bacc
import concourse.mybir as mybir
import concourse.tile as tile
from concourse.bass_utils import run_bass_kernel_spmd

F32 = mybir.dt.float32
BF16 = mybir.dt.bfloat16
I32 = mybir.dt.int32
NP_BF16 = mybir.dt.np(mybir.dt.bfloat16)
AF = mybir.ActivationFunctionType
OP = mybir.AluOpType

N_CORES = 8
B, L, D = 64, 512, 768
NF = 100
WAIST = 768
TOK = 3
CLS = 2
BL = B // N_CORES          # batch rows per core = 8
P = 128
LC = L // P                # 4 token chunks per row
DC = D // P                # 6 channel chunks
WC = WAIST // P            # 6 waist chunks
KC = DC + 1                # contraction chunks of w1 (6x128 seq + 1x100 feat)


def build_program():
    nc = bacc.Bacc("TRN2", target_bir_lowering=False, debug=False)

    seq = nc.dram_tensor("seq", [BL, L, D], F32, kind="ExternalInput").ap()
    feat = nc.dram_tensor("feat", [BL, L, NF], F32, kind="ExternalInput").ap()
    valid = nc.dram_tensor("valid", [BL, L], I32, kind="ExternalInput").ap()
    enr_w = nc.dram_tensor("enr_w", [NF, NF], BF16, kind="ExternalInput").ap()
    enr_b = nc.dram_tensor("enr_b", [NF], F32, kind="ExternalInput").ap()
    w1 = nc.dram_tensor("w1", [D + NF, WAIST], BF16, kind="ExternalInput").ap()
    b1 = nc.dram_tensor("b1", [WAIST], F32, kind="ExternalInput").ap()
    w2 = nc.dram_tensor("w2", [WAIST, TOK], BF16, kind="ExternalInput").ap()
    b2 = nc.dram_tensor("b2", [TOK], F32, kind="ExternalInput").ap()
    cls_w = nc.dram_tensor("cls_w", [D, CLS], F32, kind="ExternalInput").ap()
    cls_b = nc.dram_tensor("cls_b", [CLS], F32, kind="ExternalInput").ap()

    tag = nc.dram_tensor("tag", [BL, L, TOK], F32, kind="ExternalOutput").ap()
    cls = nc.dram_tensor("cls", [BL, CLS], F32, kind="ExternalOutput").ap()

    # NEFF-embedded constants
    ident_np = np.eye(P, dtype=np.float32)
    iota_np = np.broadcast_to(
        np.arange(L, dtype=np.float32), (P, L)
    ).copy()
    tri_np = np.triu(np.ones((L, L), dtype=np.float32))  # U[k, l] = 1 iff k <= l
    ident_d = nc.inline_tensor(ident_np, name="identc").ap()
    identb_d = nc.inline_tensor(ident_np.astype(NP_BF16), name="identbc").ap()
    iota_d = nc.inline_tensor(iota_np, name="iotac").ap()
    tri_d = nc.inline_tensor(tri_np.astype(NP_BF16), name="tric").ap()

    with tile.TileContext(nc) as tc:
        with (
            tc.tile_pool(name="const", bufs=1) as cp,
            tc.tile_pool(name="work", bufs=2) as wp,
            tc.tile_pool(name="ps_big", bufs=2, space="PSUM") as pb,
            tc.tile_pool(name="ps_small", bufs=1, space="PSUM") as ps,
        ):
            # ---- persistent constants / weights in SBUF ----
            ident = cp.tile([P, P], F32)
            nc.sync.dma_start(ident[:], ident_d[:])
            identb = cp.tile([P, P], BF16)
            nc.sync.dma_start(identb[:], identb_d[:])
            iota = cp.tile([P, L], F32)
            nc.sync.dma_start(iota[:], iota_d[:])
            tri = cp.tile([P, LC, L], BF16)
            nc.sync.dma_start(tri[:], tri_d.rearrange("(c p) l -> p c l", p=P))

            w1_sb = cp.tile([P, KC, WAIST], BF16)
            nc.sync.dma_start(
                w1_sb[:, :DC, :], w1[:D].rearrange("(c p) w -> p c w", p=P)
            )
            nc.sync.dma_start(w1_sb[:NF, DC, :], w1[D:])
            b1T = cp.tile([P, WC], F32)
            nc.sync.dma_start(b1T[:], b1.rearrange("(c p) -> p c", p=P))

            w2_sb = cp.tile([P, WC, TOK], BF16)
            nc.sync.dma_start(w2_sb[:], w2.rearrange("(c p) t -> p c t", p=P))
            b2T = cp.tile([TOK, 1], F32)
            nc.sync.dma_start(b2T[:], b2.rearrange("(t o) -> t o", o=1))

            clsw_sb = cp.tile([P, DC, CLS], F32)
            nc.sync.dma_start(clsw_sb[:], cls_w.rearrange("(c p) t -> p c t", p=P))
            clsbT = cp.tile([CLS, 1], F32)
            nc.sync.dma_start(clsbT[:], cls_b.rearrange("(t o) -> t o", o=1))

            enrw_sb = cp.tile([NF, NF], BF16)
            nc.sync.dma_start(enrw_sb[:], enr_w[:])
            enrbT = cp.tile([NF, 1], F32)
            nc.sync.dma_start(enrbT[:], enr_b.rearrange("(f o) -> f o", o=1))

            # ---- per-core prologue: destination slots tgtT[l, b] ----
            # cumsumT[l, b] = sum_k tri[k, l] * valid[b, k]  (inclusive cumsum)
            # tgtT = (cumsumT - 1) if valid else -1000
            vT_i = cp.tile([P, LC, BL], I32)
            valid_t = valid.rearrange("b (c p) -> p c b", p=P)
            for lc in range(LC):
                nc.sync.dma_start(vT_i[:, lc, :], valid_t[:, lc, :])
            vT_f = cp.tile([P, LC, BL], BF16)
            nc.vector.tensor_copy(vT_f[:], vT_i[:])

            tgtT = cp.tile([P, LC, BL], F32)
            for lc in range(LC):
                pcs = ps.tile([P, BL], F32, tag="pft")
                for kc in range(LC):
                    nc.tensor.matmul(
                        pcs[:],
                        tri[:, kc, lc * P:(lc + 1) * P],
                        vT_f[:, kc, :],
                        start=(kc == 0),
                        stop=(kc == LC - 1),
                    )
                nc.vector.tensor_scalar_add(tgtT[:, lc, :], pcs[:], 999.0)
                nc.vector.tensor_tensor(
                    out=tgtT[:, lc, :], in0=tgtT[:, lc, :], in1=vT_f[:, lc, :],
                    op=OP.mult,
                )
                nc.vector.tensor_scalar_add(tgtT[:, lc, :], tgtT[:, lc, :], -1000.0)

            # ---- cls head (once per core) ----
            clsT = cp.tile([P, DC, BL], F32)
            cls_rep_t = seq[:, 0, :].rearrange("b (c p) -> p c b", p=P)
            for dc in range(DC):
                nc.sync.dma_start(clsT[:, dc, :], cls_rep_t[:, dc, :])
            pcls = ps.tile([CLS, BL], F32, tag="penr")
            for dc in range(DC):
                nc.tensor.matmul(
                    pcls[:], clsw_sb[:, dc, :], clsT[:, dc, :],
                    start=(dc == 0), stop=(dc == DC - 1),
                )
            clsv = cp.tile([CLS, BL], F32)
            nc.scalar.activation(clsv[:], pcls[:], AF.Identity, bias=clsbT[:, 0:1])
            pct = ps.tile([BL, CLS], F32, tag="ptt")
            nc.tensor.transpose(pct[:], clsv[:], ident[:CLS, :CLS])
            clso = cp.tile([BL, CLS], F32)
            nc.vector.tensor_copy(clso[:], pct[:])
            nc.sync.dma_start(cls[:], clso[:])

            # ---- main per-row pipeline ----
            for b in range(BL):
                seq_f = wp.tile([P, LC, D], F32, tag="seqf")
                nc.sync.dma_start(seq_f[:], seq[b].rearrange("(c p) d -> p c d", p=P))
                seq_sb = wp.tile([P, LC, D], BF16, tag="seq")
                nc.vector.tensor_copy(seq_sb[:], seq_f[:])
                feat_f = wp.tile([P, LC, NF], F32, tag="featf")
                nc.sync.dma_start(feat_f[:], feat[b].rearrange("(c p) f -> p c f", p=P))
                feat_sb = wp.tile([P, LC, NF], BF16, tag="feat")
                nc.scalar.copy(feat_sb[:], feat_f[:])

                # selection matrix P^T[l, j] = (tgtT[l] == j)
                pt_sb = wp.tile([P, LC, L], BF16, tag="pt")
                for lc in range(LC):
                    nc.vector.tensor_scalar(
                        pt_sb[:, lc, :], iota[:], tgtT[:, lc, b:b + 1], None,
                        op0=OP.is_equal,
                    )

                # featT via PE transpose, then enriched^T = relu(enr_w^T @ featT)
                pft = ps.tile([NF, L], BF16, tag="pft")
                for lc in range(LC):
                    nc.tensor.transpose(
                        pft[:, lc * P:(lc + 1) * P], feat_sb[:, lc, :], identb[:]
                    )
                featT = wp.tile([NF, L], BF16, tag="featT")
                nc.vector.tensor_copy(featT[:], pft[:])

                penr = ps.tile([NF, L], F32, tag="penr")
                nc.tensor.matmul(penr[:], enrw_sb[:], featT[:], start=True, stop=True)
                enrT = wp.tile([NF, L], BF16, tag="enrT")
                nc.scalar.activation(enrT[:], penr[:], AF.Relu, bias=enrbT[:, 0:1])

                # compacted seq^T: xT[d, j] = sum_l seq[l, d] * P^T[l, j]
                xT = wp.tile([P, DC, L], BF16, tag="xT")
                for dc in range(DC):
                    px = pb.tile([P, L], F32, tag="px")
                    for lc in range(LC):
                        nc.tensor.matmul(
                            px[:],
                            seq_sb[:, lc, dc * P:(dc + 1) * P],
                            pt_sb[:, lc, :],
                            start=(lc == 0),
                            stop=(lc == LC - 1),
                        )
                    if dc % 2 == 0:
                        nc.vector.tensor_copy(xT[:, dc, :], px[:])
                    else:
                        nc.scalar.copy(xT[:, dc, :], px[:])

                # hT = relu(w1^T @ concat(xT, enrT) + b1)
                hT = wp.tile([P, WC, L], BF16, tag="hT")
                for wc in range(WC):
                    ph = pb.tile([P, L], F32, tag="ph")
                    for dc in range(DC):
                        nc.tensor.matmul(
                            ph[:],
                            w1_sb[:, dc, wc * P:(wc + 1) * P],
                            xT[:, dc, :],
                            start=(dc == 0),
                            stop=False,
                        )
                    nc.tensor.matmul(
                        ph[:],
                        w1_sb[:NF, DC, wc * P:(wc + 1) * P],
                        enrT[:],
                        start=False,
                        stop=True,
                    )
                    nc.scalar.activation(
                        hT[:, wc, :], ph[:], AF.Relu, bias=b1T[:, wc:wc + 1]
                    )

                # tagT = w2^T @ hT + b2, then transpose back to token-major
                pt_ps = ps.tile([TOK, L], F32, tag="pt_ps")
                for wc in range(WC):
                    nc.tensor.matmul(
                        pt_ps[:], w2_sb[:, wc, :], hT[:, wc, :],
                        start=(wc == 0), stop=(wc == WC - 1),
                    )
                tagT_sb = wp.tile([TOK, L], F32, tag="tagT")
                nc.scalar.activation(tagT_sb[:], pt_ps[:], AF.Identity, bias=b2T[:, 0:1])

                tag_sb = wp.tile([P, LC, TOK], F32, tag="tago")
                for lc in range(LC):
                    ptt = ps.tile([P, TOK], F32, tag="ptt")
                    nc.tensor.transpose(
                        ptt[:], tagT_sb[:, lc * P:(lc + 1) * P], ident[:TOK, :TOK]
                    )
                    nc.vector.tensor_copy(tag_sb[:, lc, :], ptt[:])
                nc.sync.dma_start(tag[b].rearrange("(c p) t -> p c t", p=P), tag_sb[:])

    nc.compile()
    return nc


_NC = None


def kernel(sequence_output, features, valid_ids, enr_w, enr_b, w1, b1, w2, b2,
           cls_w, cls_b):
    global _NC
    if _NC is None:
        _NC = build_program()

    shared = {
        "enr_w": np.ascontiguousarray(enr_w).astype(NP_BF16),
        "enr_b": np.ascontiguousarray(enr_b, np.float32),
        "w1": np.ascontiguousarray(w1).astype(NP_BF16),
        "b1": np.ascontiguousarray(b1, np.float32),
        "w2": np.ascontiguousarray(w2).astype(NP_BF16),
        "b2": np.ascontiguousarray(b2, np.float32),
        "cls_w": np.ascontiguousarray(cls_w, np.float32),
        "cls_b": np.ascontiguousarray(cls_b, np.float32),
    }
    in_maps = []
    for c in range(N_CORES):
        sl = slice(c * BL, (c + 1) * BL)
        in_maps.append({
            "seq": np.ascontiguousarray(sequence_output[sl], np.float32),
            "feat": np.ascontiguousarray(features[sl], np.float32),
            "valid": np.ascontiguousarray(valid_ids[sl], np.int32),
            **shared,
        })

    res = run_bass_kernel_spmd(_NC, in_maps, list(range(N_CORES)))
    tag = np.concatenate([res.results[c]["tag"] for c in range(N_CORES)], axis=0)
    cls = np.concatenate([res.results[c]["cls"] for c in range(N_CORES)], axis=0)
    return (cls.astype(np.float32), tag.astype(np.float32))
